# revision 28
# baseline (speedup 1.0000x reference)
"""MDTA (Restormer transposed-channel attention) Trainium2 Bass kernel.

Data-parallel over batch: 8 images -> 8 NeuronCores, one image per core.

Per-core pipeline (image = [192, 128, 128] f32):
  1. GEMM1 (1x1 conv): qkv = w_qkv @ x on PE, bf16 in / f32 psum, written
     into a row-padded buffer (row stride 130) so the depthwise conv reads
     zeros at image edges. Channels permuted into 5 partition tiles:
     [q0:128, k0:128, v0:128, q128:192|k128:192, v128:192].
  2. Depthwise 3x3: 9 accumulating PE matmuls per 4-row tile with diagonal
     stationaries; tap shift via rhs AP offset into the padded buffer.
  3. q/k transposed chunk-wise with 4 DMA-transpose (xbar) instructions;
     logits qT.T@kT accumulate into persistent PSUM; L2-norm sums via ACT
     Square+accum_out. v's depthwise runs after the transposes are issued
     so PE stays busy while the xbar works.
  4. Softmax on rsqrt-scaled logits (block-diag mask per head), then the
     projection is folded into attention: FT = (attn.T @ wp) so that
     out = FT.T @ v is a single GEMM pass streamed straight to HBM (bf16).

All matmuls are zero-padded to K=128/M=128 (padding rows/cols cost nothing:
stream time depends only on N) because the PE only pipelines LDWEIGHTS under
the previous matmul when consecutive matmuls keep the same array geometry.
Filler matmuls with staged dependencies keep the PE HAM clock warm through
the softmax phase so pass 2 runs at 2.4 GHz instead of 1.2 GHz.
"""

import os
import sys
import numpy as np

for _p in ("/opt/trn_rl_repo",):
    if _p not in sys.path and os.path.isdir(_p):
        sys.path.insert(0, _p)

import ml_dtypes

BF16 = ml_dtypes.bfloat16

B, C, H, W = 8, 192, 128, 128
HEADS, DH = 4, 48
C3 = 3 * C            # 576
C3P = 640             # padded (5 x 128 output-channel tiles)
HW = H * W            # 16384
RT = W + 2            # padded row stride = 130
TY = 16               # image rows per chunk
NCHUNK = H // TY      # 8
SLOTS = TY + 2        # 18 row slots per chunk (halo)
# partition tiles over the PERMUTED 576 channels
PT5 = [128, 128, 128, 128, 64]
PO5 = [0, 128, 256, 384, 512]

_CACHE = {}


def _build():
    import concourse.bass as bass
    import concourse.bacc as bacc
    import concourse.tile as tile
    import concourse.mybir as mybir

    f32 = mybir.dt.float32
    bf16 = mybir.dt.bfloat16
    Alu = mybir.AluOpType
    Act = mybir.ActivationFunctionType

    nc = bacc.Bacc("TRN2", target_bir_lowering=False, debug=False,
                   enable_asserts=False)

    x_d = nc.dram_tensor("x0", [C, H, W], bf16, kind="ExternalInput").ap()
    wqa_d = nc.dram_tensor("wqa", [128, C3P], bf16, kind="ExternalInput").ap()
    wqb_d = nc.dram_tensor("wqb", [128, C3P], bf16, kind="ExternalInput").ap()
    wdg_d = nc.dram_tensor("wdg", [128, 45 * 128], bf16, kind="ExternalInput").ap()
    wpa_d = nc.dram_tensor("wpa", [128, C], bf16, kind="ExternalInput").ap()
    wpb_d = nc.dram_tensor("wpb", [128, C], bf16, kind="ExternalInput").ap()
    tv_d = nc.dram_tensor("tv", [C, 1], f32, kind="ExternalInput").ap()
    idf_d = nc.dram_tensor("idf", [128, 128], f32, kind="ExternalInput").ap()
    ones_d = nc.dram_tensor("ones1", [1, 128], f32, kind="ExternalInput").ap()
    mka_d = nc.dram_tensor("mka", [128, C], f32, kind="ExternalInput").ap()
    mkb_d = nc.dram_tensor("mkb", [64, C], f32, kind="ExternalInput").ap()
    out_d = nc.dram_tensor("out0", [C, H, W], bf16, kind="ExternalOutput").ap()

    with tile.TileContext(nc) as tc:
        with (
            tc.tile_pool(name="const", bufs=1) as cpool,
            tc.tile_pool(name="big", bufs=1) as big,
            tc.tile_pool(name="xin", bufs=2) as xpool,
            tc.tile_pool(name="qk", bufs=2) as qkpool,
            tc.tile_pool(name="qt", bufs=1) as qtpool,
            tc.tile_pool(name="small", bufs=1) as small,
        ):
            # ---- persistent big buffers ----
            qkv = big.tile([128, 5, SLOTS, RT], bf16, tag="qkv")
            v_a = big.tile([128, H, W], bf16, tag="va")
            v_b = big.tile([128, H, W], bf16, tag="vb")   # rows 64:128 stay 0

            wq_a = cpool.tile([128, C3P], bf16, tag="wqa")
            wq_b = cpool.tile([128, C3P], bf16, tag="wqb")  # rows 64:128 zero
            wdg = cpool.tile([128, 45, 128], bf16, tag="wdg")
            wp_a = cpool.tile([128, C], bf16, tag="wpa")
            wp_b = cpool.tile([128, C], bf16, tag="wpb")    # rows 64:128 zero
            idf = cpool.tile([128, 128], f32, tag="idf")
            ones1 = cpool.tile([1, 128], f32, tag="ones1")
            tv_a = cpool.tile([128, 1], f32, tag="tva")
            tv_b = cpool.tile([64, 1], f32, tag="tvb")
            mk_a = cpool.tile([128, C], f32, tag="mka")
            mk_b = cpool.tile([64, C], f32, tag="mkb")
            # explicit double buffer for xb so the zero padding (rows 64:128)
            # survives across chunks
            xb0 = cpool.tile([128, SLOTS, W], bf16, tag="xb0")
            xb1 = cpool.tile([128, SLOTS, W], bf16, tag="xb1")

            # first x chunk starts the critical path on the SP HWDGE ring;
            # weights go down the ACT HWDGE ring in parallel
            xa0 = xpool.tile([128, SLOTS, W], bf16, tag="xa")
            nc.vector.memset(xa0[:, 0, :], 0.0)
            nc.vector.memset(xb0[0:64, 0, :], 0.0)
            nc.sync.dma_start(xa0[:, 1:SLOTS, :], x_d[0:128, 0:SLOTS - 1, :])
            nc.sync.dma_start(xb0[0:64, 1:SLOTS, :], x_d[128:192, 0:SLOTS - 1, :])
            nc.scalar.dma_start(wq_a[:], wqa_d[:])
            nc.scalar.dma_start(wq_b[:], wqb_d[:])
            nc.scalar.dma_start(wdg[:], wdg_d[:, :])
            nc.scalar.dma_start(wp_a[:], wpa_d[:])
            nc.scalar.dma_start(wp_b[:], wpb_d[:])
            nc.scalar.dma_start(idf[:], idf_d[:])
            nc.scalar.dma_start(ones1[:], ones_d[:])
            nc.scalar.dma_start(tv_a[:], tv_d[0:128, :])
            nc.scalar.dma_start(tv_b[:], tv_d[128:192, :])
            nc.scalar.dma_start(mk_a[:], mka_d[:])
            nc.scalar.dma_start(mk_b[:], mkb_d[:])

            # only the pad columns / pad partitions of qkv need zeroing
            # (GEMM1 rewrites every data column each chunk)
            nc.vector.memset(qkv[:, :, :, 0:RT:RT - 1], 0.0)
            # zero pads on GpSimd (otherwise idle); xb pads first — chunk-0
            # GEMM1 depends on them. v_b's big pad isn't read until pass 2.
            nc.gpsimd.memset(xb0[64:128, :, :], 0.0)
            nc.gpsimd.memset(xb1[64:128, :, :], 0.0)
            nc.gpsimd.memset(qkv[64:128, 4, :, :], 0.0)
            nc.gpsimd.memset(v_b[64:128, :, :], 0.0)

            # l2 norm partial sums per chunk, packed for a short rsqrt chain:
            # ssA free-dim-packs q_a/k_a sums; ssB partition-packs q_b (0:64)
            # and k_b (64:128) sums
            ssA = small.tile([128, 2, NCHUNK], f32, tag="ssA")
            ssB = small.tile([128, NCHUNK], f32, tag="ssB")

            sqs = big.tile([128, TY, W], bf16, tag="sqs")
            qT = qtpool.tile([128, TY, C], bf16, tag="qT")
            kT = qtpool.tile([128, TY, C], bf16, tag="kT")

            # padded tail tensors (pads zeroed once, on GpSimd)
            attn_a = small.tile([128, 256], bf16, tag="atta")
            attn_b = small.tile([128, 256], bf16, tag="attb")
            FT_a = small.tile([128, 256], bf16, tag="FTa")
            FT_b = small.tile([128, 256], bf16, tag="FTb")
            nc.gpsimd.memset(attn_a[:, C:256], 0.0)
            nc.gpsimd.memset(attn_b[:], 0.0)
            nc.gpsimd.memset(FT_a[:], 0.0)
            nc.gpsimd.memset(FT_b[:], 0.0)

            ncop = [0]

            def evac(dst, src):
                # alternate PSUM evacuation between DVE and ACT
                if ncop[0] % 2 == 0:
                    nc.vector.tensor_copy(dst, src)
                else:
                    nc.scalar.copy(dst, src)
                ncop[0] += 1

            with (
                tc.tile_pool(name="work", bufs=6, space="PSUM") as work,
                tc.tile_pool(name="psl", bufs=1, space="PSUM") as psl,
            ):
                lg_a = psl.tile([128, C], f32, tag="lga")
                lg_b = psl.tile([64, C], f32, tag="lgb")

                # pre-warm the PE HAM clock during the startup DMA wait;
                # keyed off the x DMA, which lands before the weights
                junk0 = work.tile([128, 512], f32, tag="w")
                for _ in range(8):
                    nc.tensor.matmul(junk0[:, :], xa0[:, 1, 0:128],
                                     xa0[:, 1:5, :],
                                     start=True, stop=True, skip_group_check=True)

                pending_squares = None
                for ch in range(NCHUNK):
                    y0 = ch * TY
                    # ---- load x chunk (rows y0-1 .. y0+16) ----
                    if ch == 0:
                        xa, xb = xa0, xb0
                    else:
                        xa = xpool.tile([128, SLOTS, W], bf16, tag="xa")
                        xb = xb0 if ch % 2 == 0 else xb1
                        lo = y0 - 1
                        hi = min(y0 + TY, H - 1)
                        s1 = hi - (y0 - 1)
                        if ch == NCHUNK - 1:
                            nc.vector.memset(xa[:, SLOTS - 1, :], 0.0)
                            nc.vector.memset(xb[0:64, SLOTS - 1, :], 0.0)
                        nc.sync.dma_start(xa[:, 0:s1 + 1, :], x_d[0:128, lo:hi + 1, :])
                        nc.sync.dma_start(xb[0:64, 0:s1 + 1, :],
                                          x_d[128:192, lo:hi + 1, :])

                    # ---- GEMM1: qkv = wq.T @ x, 3 rows (384 cols) at a time ----
                    for r in range(5):
                        po, pn = 128 * r, PT5[r]
                        for j in range(SLOTS // 3):
                            pg = work.tile([128, 3, W], f32, tag="w")
                            nc.tensor.matmul(
                                pg[:, :, :], wq_a[:, po:po + 128],
                                xa[:, 3 * j:3 * j + 3, :], start=True, stop=False)
                            nc.tensor.matmul(
                                pg[:, :, :], wq_b[:, po:po + 128],
                                xb[:, 3 * j:3 * j + 3, :], start=False, stop=True)
                            evac(qkv[0:pn, r, 3 * j:3 * j + 3, 1:1 + W],
                                 pg[0:pn, :, :])

                    # ---- previous chunk's L2-norm squares run in ACT's slack ----
                    if pending_squares is not None:
                        pending_squares()
                        pending_squares = None

                    # ---- depthwise 3x3 for q,k tiles (r = 0, 1, 3) ----
                    q_a = qkpool.tile([128, TY, W], bf16, tag="qa")
                    k_a = qkpool.tile([128, TY, W], bf16, tag="ka")
                    qk_b = qkpool.tile([128, TY, W], bf16, tag="qkb")
                    dstmap = {0: q_a, 1: k_a, 3: qk_b}
                    for r in (0, 1, 3):
                        for t in range(TY // 4):
                            pd = work.tile([128, 4, W], f32, tag="w")
                            for kk in range(9):
                                dy, dx = kk // 3 - 1, kk % 3 - 1
                                srow = 1 + 4 * t + dy
                                nc.tensor.matmul(
                                    pd[:, :, :],
                                    wdg[:, r * 9 + kk, :],
                                    qkv[:, r, srow:srow + 4, 1 + dx:1 + dx + W],
                                    start=(kk == 0), stop=(kk == 8))
                            d = dstmap[r]
                            evac(d[:, 4 * t:4 * t + 4, :], pd[:, :, :])

                    # ---- L2 norm partial sums (ACT: square + accum), deferred
                    #      one chunk so GEMM1's evac copies aren't queued
                    #      behind them on ACT ----
                    def make_squares(q_a=q_a, k_a=k_a, qk_b=qk_b, ch=ch):
                        def emit():
                            nc.scalar.activation(sqs[:, :, :], q_a[:, :, :],
                                                 Act.Square,
                                                 accum_out=ssA[:, 0, ch:ch + 1])
                            nc.scalar.activation(sqs[0:64, :, :], qk_b[0:64, :, :],
                                                 Act.Square,
                                                 accum_out=ssB[0:64, ch:ch + 1])
                            nc.scalar.activation(sqs[:, :, :], k_a[:, :, :],
                                                 Act.Square,
                                                 accum_out=ssA[:, 1, ch:ch + 1])
                            nc.scalar.activation(sqs[64:128, :, :],
                                                 qk_b[64:128, :, :], Act.Square,
                                                 accum_out=ssB[64:128, ch:ch + 1])
                        return emit
                    if ch == NCHUNK - 1:
                        # last chunk: run now so the sums are ready when the
                        # softmax chain starts (overlaps dw-v / logits)
                        make_squares()()
                        pending_squares = None
                    else:
                        pending_squares = make_squares()

                    # ---- chunk-wise q/k transpose on the DMA xbar ----
                    nc.sync.dma_start(qT[:, :, 0:128], q_a[:, :, :], transpose=True)
                    nc.sync.dma_start(qT[:, :, 128:192], qk_b[0:64, :, :], transpose=True)
                    nc.sync.dma_start(kT[:, :, 0:128], k_a[:, :, :], transpose=True)
                    nc.sync.dma_start(kT[:, :, 128:192], qk_b[64:128, :, :], transpose=True)

                    # ---- depthwise for v tiles (r = 2, 4); overlaps the xbar ----
                    for r in (2, 4):
                        for t in range(TY // 4):
                            pd = work.tile([128, 4, W], f32, tag="w")
                            for kk in range(9):
                                dy, dx = kk // 3 - 1, kk % 3 - 1
                                srow = 1 + 4 * t + dy
                                nc.tensor.matmul(
                                    pd[:, :, :],
                                    wdg[:, r * 9 + kk, :],
                                    qkv[:, r, srow:srow + 4, 1 + dx:1 + dx + W],
                                    start=(kk == 0), stop=(kk == 8))
                            dst = y0 + 4 * t
                            if r == 2:
                                evac(v_a[:, dst:dst + 4, :], pd[:, :, :])
                            else:
                                evac(v_b[0:64, dst:dst + 4, :], pd[0:64, :, :])

                    # ---- accumulate logits (grouped by M so the PE array
                    #      geometry stays constant within each run) ----
                    for s in range(TY):
                        first = (ch == 0 and s == 0)
                        last = (ch == NCHUNK - 1 and s == TY - 1)
                        nc.tensor.matmul(lg_a[:, :], qT[:, s, 0:128], kT[:, s, :],
                                         start=first, stop=last, skip_group_check=True)
                    for s in range(TY):
                        first = (ch == 0 and s == 0)
                        last = (ch == NCHUNK - 1 and s == TY - 1)
                        nc.tensor.matmul(lg_b[:, :], qT[:, s, 128:192], kT[:, s, :],
                                         start=first, stop=last, skip_group_check=True)

                if pending_squares is not None:
                    pending_squares()
                    pending_squares = None

                # PE warm-keeper: junk matmuls with staged deps on the softmax
                # chain so the HAM clock never sees a ~3.4us idle window.
                junk = work.tile([128, 512], f32, tag="w")
                sqs_f32 = sqs[:, :, :].bitcast(f32)   # junk rhs, N=512

                def filler(dep):
                    m = dep.free_size()
                    nc.tensor.matmul(junk[0:m, :], dep, sqs_f32[:, 0:8, :],
                                     start=True, stop=True, skip_group_check=True)

                # ================= softmax prep =================
                # rrA[:, 0] = rq (q lo/tv-scaled), rrA[:, 1] = rk (k lo);
                # rrB[0:64] = rq hi, rrB[64:128] = rk hi
                rrA = small.tile([128, 2], f32, tag="rrA")
                rrB = small.tile([128, 1], f32, tag="rrB")
                tmpA = small.tile([128, 2], f32, tag="tmpA")
                tmpB = small.tile([128, 1], f32, tag="tmpB")
                nc.vector.tensor_reduce(tmpA[:], ssA[:, :, :],
                                        mybir.AxisListType.X, Alu.add)
                nc.vector.tensor_reduce(tmpB[:], ssB[:, :],
                                        mybir.AxisListType.X, Alu.add)
                nc.scalar.activation(tmpA[:], tmpA[:], Act.Sqrt)
                nc.scalar.activation(tmpB[:], tmpB[:], Act.Sqrt)
                nc.vector.tensor_scalar_max(tmpA[:], tmpA[:], 1e-12)
                nc.vector.tensor_scalar_max(tmpB[:], tmpB[:], 1e-12)
                nc.vector.reciprocal(rrA[:], tmpA[:])
                nc.vector.reciprocal(rrB[:], tmpB[:])
                filler(tmpA[:, :])
                nc.vector.tensor_tensor(rrA[:, 0:1], rrA[:, 0:1], tv_a[:], Alu.mult)
                nc.vector.tensor_tensor(rrB[0:64, :], rrB[0:64, :], tv_b[:], Alu.mult)
                filler(rrB[:, :])
                rq_a, rk_a = rrA[:, 0:1], rrA[:, 1:2]
                rq_b, rk_b = rrB[0:64, :], rrB[64:128, :]

                # copy logits out of psum, scale rows by rq
                L_a = small.tile([128, C], f32, tag="La")
                L_b = small.tile([64, C], f32, tag="Lb")
                nc.vector.tensor_scalar(L_a[:], lg_a[:], rq_a, None, Alu.mult)
                nc.vector.tensor_scalar(L_b[:], lg_b[:], rq_b, None, Alu.mult)
                filler(L_a[:, 0:128])

            with tc.tile_pool(name="psx", bufs=1, space="PSUM") as psx:
                junk2 = psx.tile([128, 512], f32, tag="junk2")

                def filler2(dep):
                    m = dep.free_size()
                    nc.tensor.matmul(junk2[0:m, :], dep, sqs_f32[:, 0:8, :],
                                     start=True, stop=True, skip_group_check=True)

                # column scale: bcast rk over partitions via K=1 matmul
                rkrow = small.tile([1, C], f32, tag="rkrow")
                pb = psx.tile([128, C], f32, tag="pb")
                nc.tensor.transpose(pb[0:1, 0:128], rk_a, idf[:])
                nc.tensor.transpose(pb[0:1, 128:192], rk_b,
                                    idf[64:128, 64:128])
                nc.any.tensor_copy(rkrow[:], pb[0:1, 0:192])
                pbc = psx.tile([128, C], f32, tag="pbc")
                nc.tensor.matmul(pbc[:, :], ones1[:], rkrow[:], start=True, stop=True)
                nc.vector.tensor_tensor(L_a[:], L_a[:], pbc[:, :], Alu.mult)
                nc.vector.tensor_tensor(L_b[:], L_b[:], pbc[0:64, :], Alu.mult)
                filler2(L_a[:, 0:128])

                # full-row softmax; cross-head blocks masked to -1e30 -> exp 0.
                # a/b halves use independent scratch so their DVE/ACT ops
                # interleave instead of serializing.
                mx = small.tile([128, 1], f32, tag="mx")
                sm = small.tile([128, 1], f32, tag="sm")
                E = small.tile([128, C], f32, tag="E")
                mx2 = small.tile([64, 1], f32, tag="mx2")
                sm2 = small.tile([64, 1], f32, tag="sm2")
                E2 = small.tile([64, C], f32, tag="E2")
                nc.vector.tensor_tensor(L_a[:], L_a[:], mk_a[:], Alu.add)
                nc.vector.tensor_tensor(L_b[:], L_b[:], mk_b[:], Alu.add)
                nc.vector.tensor_reduce(mx[:], L_a[:], mybir.AxisListType.X, Alu.max)
                nc.vector.tensor_reduce(mx2[:], L_b[:], mybir.AxisListType.X, Alu.max)
                nc.vector.tensor_scalar_mul(mx[:], mx[:], -1.0)
                nc.vector.tensor_scalar_mul(mx2[:], mx2[:], -1.0)
                filler2(mx[:, :])
                nc.scalar.activation(E[:, :], L_a[:], Act.Exp,
                                     bias=mx[:], accum_out=sm[:])
                nc.scalar.activation(E2[:, :], L_b[:], Act.Exp,
                                     bias=mx2[:], accum_out=sm2[:])
                nc.vector.reciprocal(sm[:], sm[:])
                nc.vector.reciprocal(sm2[:], sm2[:])
                filler2(E[:, 0:128])
                nc.scalar.activation(attn_a[:, 0:C], E[:, :], Act.Copy,
                                     scale=sm[:])
                nc.scalar.activation(attn_b[0:64, 0:C], E2[:, :], Act.Copy,
                                     scale=sm2[:])
                filler2(E[:, 0:128])

                # fold projection into attention: FT[d, o] = sum_c attn[c,d] wp[c,o]
                pfa = psx.tile([128, C], f32, tag="pfa")
                pfb = psx.tile([128, C], f32, tag="pfb")
                nc.tensor.matmul(pfa[:, :], attn_a[:, 0:128], wp_a[:, :],
                                 start=True, stop=False)
                nc.tensor.matmul(pfa[:, :], attn_b[:, 0:128], wp_b[:, :],
                                 start=False, stop=True)
                nc.tensor.matmul(pfb[:, :], attn_a[:, 128:256], wp_a[:, :],
                                 start=True, stop=False)
                nc.tensor.matmul(pfb[:, :], attn_b[:, 128:256], wp_b[:, :],
                                 start=False, stop=True)
                nc.vector.tensor_copy(FT_a[:, 0:C], pfa[:, :])
                nc.scalar.copy(FT_b[0:64, 0:C], pfb[0:64, :])

            # ---- pass 2: out = FT.T @ v, streamed to HBM in bf16.
            #      Output staged in 8-row tiles so each DMA moves 256 KB. ----
            TB = 8
            with (
                tc.tile_pool(name="pout", bufs=4, space="PSUM") as pout,
                tc.tile_pool(name="o2", bufs=3) as opool,
            ):
                for blk in range(H // TB):
                    yb = blk * TB
                    ot_a = opool.tile([128, TB, W], bf16, tag="ota")
                    ot_b = opool.tile([64, TB, W], bf16, tag="otb")
                    for t in range(TB // 4):
                        r4 = yb + 4 * t
                        poa = pout.tile([128, 4, W], f32, tag="poa")
                        pob = pout.tile([128, 4, W], f32, tag="pob")
                        nc.tensor.matmul(poa[:, :, :], FT_a[:, 0:128],
                                         v_a[:, r4:r4 + 4, :], start=True, stop=False)
                        nc.tensor.matmul(poa[:, :, :], FT_b[:, 0:128],
                                         v_b[:, r4:r4 + 4, :], start=False, stop=True)
                        nc.tensor.matmul(pob[:, :, :], FT_a[:, 128:256],
                                         v_a[:, r4:r4 + 4, :], start=True, stop=False)
                        nc.tensor.matmul(pob[:, :, :], FT_b[:, 128:256],
                                         v_b[:, r4:r4 + 4, :], start=False, stop=True)
                        evac(ot_a[:, 4 * t:4 * t + 4, :], poa[:, :, :])
                        evac(ot_b[:, 4 * t:4 * t + 4, :], pob[0:64, :, :])
                    nc.sync.dma_start(out_d[0:128, yb:yb + TB, :], ot_a[:])
                    nc.sync.dma_start(out_d[128:192, yb:yb + TB, :], ot_b[:])

    nc.compile()
    return nc


# permuted channel order: tile0=q[0:128], tile1=k[0:128], tile2=v[0:128],
# tile3=q[128:192]+k[128:192], tile4=v[128:192]  (orig rows of w_qkv)
def _perm():
    return np.concatenate([
        np.arange(0, 128),          # q lo
        np.arange(192, 320),        # k lo
        np.arange(384, 512),        # v lo
        np.arange(128, 192),        # q hi
        np.arange(320, 384),        # k hi
        np.arange(512, 576),        # v hi
    ])


def _prep_weights(w_qkv, w_dw, w_project, temperature):
    perm = _perm()
    wqt = w_qkv[perm].T                                          # [192, 576]
    # pad to [128, 640]: wqa = input ch 0:128; wqb = input ch 128:192 + zeros
    wqa = np.zeros((128, C3P), np.float32)
    wqb = np.zeros((128, C3P), np.float32)
    for r in range(5):
        pn = PT5[r]
        wqa[:, 128 * r:128 * r + pn] = wqt[0:128, PO5[r]:PO5[r] + pn]
        wqb[0:64, 128 * r:128 * r + pn] = wqt[128:192, PO5[r]:PO5[r] + pn]
    # diagonal stationaries: block (r*9+k) = diag(w_dw[perm ch, k])
    wdg = np.zeros((128, 45, 128), np.float32)
    wd = w_dw.reshape(C3, 9)[perm]
    for r in range(5):
        po, pn = PO5[r], PT5[r]
        for k in range(9):
            blk = wdg[:, r * 9 + k, :]
            blk[np.arange(pn), np.arange(pn)] = wd[po:po + pn, k]
    wpt = w_project.T                                            # [c, o]
    wpa = wpt[0:128]
    wpb = np.zeros((128, C), np.float32)
    wpb[0:64] = wpt[128:192]
    tv = np.repeat(temperature.reshape(HEADS), DH).reshape(C, 1).astype(np.float32)
    mk = np.full((C, C), -1e30, np.float32)
    for h in range(HEADS):
        mk[h * DH:(h + 1) * DH, h * DH:(h + 1) * DH] = 0.0
    return {
        "wqa": wqa.astype(BF16),
        "wqb": wqb.astype(BF16),
        "wpa": np.ascontiguousarray(wpa).astype(BF16),
        "wpb": wpb.astype(BF16),
        "wdg": wdg.reshape(128, 45 * 128).astype(BF16),
        "tv": tv,
        "idf": np.eye(128, dtype=np.float32),
        "ones1": np.ones((1, 128), np.float32),
        "mka": mk[0:128],
        "mkb": mk[128:192],
    }


def kernel(x, w_qkv, w_dw, w_project, temperature, heads):
    from concourse import bass_utils

    x = np.asarray(x, np.float32)
    key = "nc"
    if key not in _CACHE:
        _CACHE[key] = _build()
    nc = _CACHE[key]

    shared = _prep_weights(np.asarray(w_qkv, np.float32),
                           np.asarray(w_dw, np.float32),
                           np.asarray(w_project, np.float32),
                           np.asarray(temperature, np.float32))
    in_maps = []
    for i in range(B):
        m = dict(shared)
        m["x0"] = x[i].reshape(C, H, W).astype(BF16)
        in_maps.append(m)

    res = bass_utils.run_bass_kernel_spmd(nc, in_maps, core_ids=list(range(B)))
    outs = [r["out0"].reshape(C, H, W) for r in res.results]
    return np.stack(outs, axis=0).astype(np.float32)


if __name__ == "__main__":
    rng = np.random.default_rng(0)
    x = rng.standard_normal((B, C, H, W)).astype(np.float32)
    w_qkv = (rng.standard_normal((C3, C)) / np.sqrt(C)).astype(np.float32)
    w_dw = (rng.standard_normal((C3, 1, 3, 3)) / 3.0).astype(np.float32)
    w_project = (rng.standard_normal((C, C)) / np.sqrt(C)).astype(np.float32)
    temperature = np.ones((HEADS, 1, 1), np.float32)
    y = kernel(x=x, w_qkv=w_qkv, w_dw=w_dw, w_project=w_project,
               temperature=temperature, heads=HEADS)
    print(y.shape, y.dtype)


# revision 29
# speedup vs baseline: 1.0112x; 1.0112x over previous
"""MDTA (Restormer transposed-channel attention) Trainium2 Bass kernel.

Data-parallel over batch: 8 images -> 8 NeuronCores, one image per core.

Per-core pipeline (image = [192, 128, 128] f32):
  1. GEMM1 (1x1 conv): qkv = w_qkv @ x on PE, bf16 in / f32 psum, written
     into a row-padded buffer (row stride 130) so the depthwise conv reads
     zeros at image edges. Channels permuted into 5 partition tiles:
     [q0:128, k0:128, v0:128, q128:192|k128:192, v128:192].
  2. Depthwise 3x3: 9 accumulating PE matmuls per 4-row tile with diagonal
     stationaries; tap shift via rhs AP offset into the padded buffer.
  3. q/k transposed chunk-wise with 4 DMA-transpose (xbar) instructions;
     logits qT.T@kT accumulate into persistent PSUM; L2-norm sums via ACT
     Square+accum_out. v's depthwise runs after the transposes are issued
     so PE stays busy while the xbar works.
  4. Softmax on rsqrt-scaled logits (block-diag mask per head), then the
     projection is folded into attention: FT = (attn.T @ wp) so that
     out = FT.T @ v is a single GEMM pass streamed straight to HBM (bf16).

All matmuls are zero-padded to K=128/M=128 (padding rows/cols cost nothing:
stream time depends only on N) because the PE only pipelines LDWEIGHTS under
the previous matmul when consecutive matmuls keep the same array geometry.
Filler matmuls with staged dependencies keep the PE HAM clock warm through
the softmax phase so pass 2 runs at 2.4 GHz instead of 1.2 GHz.
"""

import os
import sys
import numpy as np

for _p in ("/opt/trn_rl_repo",):
    if _p not in sys.path and os.path.isdir(_p):
        sys.path.insert(0, _p)

import ml_dtypes

BF16 = ml_dtypes.bfloat16

B, C, H, W = 8, 192, 128, 128
HEADS, DH = 4, 48
C3 = 3 * C            # 576
C3P = 640             # padded (5 x 128 output-channel tiles)
HW = H * W            # 16384
RT = W + 2            # padded row stride = 130
TY = 16               # image rows per chunk
NCHUNK = H // TY      # 8
SLOTS = TY + 2        # 18 row slots per chunk (halo)
# partition tiles over the PERMUTED 576 channels
PT5 = [128, 128, 128, 128, 64]
PO5 = [0, 128, 256, 384, 512]

_CACHE = {}


def _build():
    import concourse.bass as bass
    import concourse.bacc as bacc
    import concourse.tile as tile
    import concourse.mybir as mybir

    f32 = mybir.dt.float32
    bf16 = mybir.dt.bfloat16
    Alu = mybir.AluOpType
    Act = mybir.ActivationFunctionType

    nc = bacc.Bacc("TRN2", target_bir_lowering=False, debug=False,
                   enable_asserts=False)

    x_d = nc.dram_tensor("x0", [C, H, W], bf16, kind="ExternalInput").ap()
    wqa_d = nc.dram_tensor("wqa", [128, C3P], bf16, kind="ExternalInput").ap()
    wqb_d = nc.dram_tensor("wqb", [128, C3P], bf16, kind="ExternalInput").ap()
    wdg_d = nc.dram_tensor("wdg", [128, 45 * 128], bf16, kind="ExternalInput").ap()
    wpa_d = nc.dram_tensor("wpa", [128, C], bf16, kind="ExternalInput").ap()
    wpb_d = nc.dram_tensor("wpb", [128, C], bf16, kind="ExternalInput").ap()
    tv_d = nc.dram_tensor("tv", [C, 1], f32, kind="ExternalInput").ap()
    idf_d = nc.dram_tensor("idf", [128, 128], f32, kind="ExternalInput").ap()
    ones_d = nc.dram_tensor("ones1", [1, 128], f32, kind="ExternalInput").ap()
    mka_d = nc.dram_tensor("mka", [128, C], f32, kind="ExternalInput").ap()
    mkb_d = nc.dram_tensor("mkb", [64, C], f32, kind="ExternalInput").ap()
    out_d = nc.dram_tensor("out0", [C, H, W], bf16, kind="ExternalOutput").ap()

    with tile.TileContext(nc) as tc:
        with (
            tc.tile_pool(name="const", bufs=1) as cpool,
            tc.tile_pool(name="big", bufs=1) as big,
            tc.tile_pool(name="xin", bufs=2) as xpool,
            tc.tile_pool(name="qk", bufs=2) as qkpool,
            tc.tile_pool(name="qt", bufs=1) as qtpool,
            tc.tile_pool(name="small", bufs=1) as small,
        ):
            # ---- persistent big buffers ----
            qkv = big.tile([128, 5, SLOTS, RT], bf16, tag="qkv")
            v_a = big.tile([128, H, W], bf16, tag="va")
            v_b = big.tile([128, H, W], bf16, tag="vb")   # rows 64:128 stay 0

            wq_a = cpool.tile([128, C3P], bf16, tag="wqa")
            wq_b = cpool.tile([128, C3P], bf16, tag="wqb")  # rows 64:128 zero
            wdg = cpool.tile([128, 45, 128], bf16, tag="wdg")
            wp_a = cpool.tile([128, C], bf16, tag="wpa")
            wp_b = cpool.tile([128, C], bf16, tag="wpb")    # rows 64:128 zero
            idf = cpool.tile([128, 128], f32, tag="idf")
            ones1 = cpool.tile([1, 128], f32, tag="ones1")
            tv_a = cpool.tile([128, 1], f32, tag="tva")
            tv_b = cpool.tile([64, 1], f32, tag="tvb")
            mk_a = cpool.tile([128, C], f32, tag="mka")
            mk_b = cpool.tile([64, C], f32, tag="mkb")
            # explicit double buffer for xb so the zero padding (rows 64:128)
            # survives across chunks
            xb0 = cpool.tile([128, SLOTS, W], bf16, tag="xb0")
            xb1 = cpool.tile([128, SLOTS, W], bf16, tag="xb1")

            # first x chunk starts the critical path on the SP HWDGE ring;
            # weights go down the ACT HWDGE ring in parallel
            xa0 = xpool.tile([128, SLOTS, W], bf16, tag="xa")
            nc.vector.memset(xa0[:, 0, :], 0.0)
            nc.vector.memset(xb0[0:64, 0, :], 0.0)
            nc.sync.dma_start(xa0[:, 1:SLOTS, :], x_d[0:128, 0:SLOTS - 1, :])
            nc.sync.dma_start(xb0[0:64, 1:SLOTS, :], x_d[128:192, 0:SLOTS - 1, :])
            nc.sync.dma_start(wq_a[:], wqa_d[:])
            nc.sync.dma_start(wq_b[:], wqb_d[:])
            nc.sync.dma_start(wdg[:], wdg_d[:, :])
            nc.scalar.dma_start(wp_a[:], wpa_d[:])
            nc.scalar.dma_start(wp_b[:], wpb_d[:])
            nc.scalar.dma_start(idf[:], idf_d[:])
            nc.scalar.dma_start(ones1[:], ones_d[:])
            nc.scalar.dma_start(tv_a[:], tv_d[0:128, :])
            nc.scalar.dma_start(tv_b[:], tv_d[128:192, :])
            nc.scalar.dma_start(mk_a[:], mka_d[:])
            nc.scalar.dma_start(mk_b[:], mkb_d[:])

            # only the pad columns / pad partitions of qkv need zeroing
            # (GEMM1 rewrites every data column each chunk)
            nc.vector.memset(qkv[:, :, :, 0:RT:RT - 1], 0.0)
            # zero pads on GpSimd (otherwise idle); xb pads first — chunk-0
            # GEMM1 depends on them. v_b's big pad isn't read until pass 2.
            nc.gpsimd.memset(xb0[64:128, :, :], 0.0)
            nc.gpsimd.memset(xb1[64:128, :, :], 0.0)
            nc.gpsimd.memset(qkv[64:128, 4, :, :], 0.0)
            nc.gpsimd.memset(v_b[64:128, :, :], 0.0)

            # l2 norm partial sums per chunk, packed for a short rsqrt chain:
            # ssA free-dim-packs q_a/k_a sums; ssB partition-packs q_b (0:64)
            # and k_b (64:128) sums
            ssA = small.tile([128, 2, NCHUNK], f32, tag="ssA")
            ssB = small.tile([128, NCHUNK], f32, tag="ssB")

            sqs = big.tile([128, TY, W], bf16, tag="sqs")
            qT = qtpool.tile([128, TY, C], bf16, tag="qT")
            kT = qtpool.tile([128, TY, C], bf16, tag="kT")

            # padded tail tensors (pads zeroed once, on GpSimd)
            attn_a = small.tile([128, 256], bf16, tag="atta")
            attn_b = small.tile([128, 256], bf16, tag="attb")
            FT_a = small.tile([128, 256], bf16, tag="FTa")
            FT_b = small.tile([128, 256], bf16, tag="FTb")
            nc.gpsimd.memset(attn_a[:, C:256], 0.0)
            nc.gpsimd.memset(attn_b[:], 0.0)
            nc.gpsimd.memset(FT_a[:], 0.0)
            nc.gpsimd.memset(FT_b[:], 0.0)

            ncop = [0]

            def evac(dst, src):
                # alternate PSUM evacuation between DVE and ACT
                if ncop[0] % 2 == 0:
                    nc.vector.tensor_copy(dst, src)
                else:
                    nc.scalar.copy(dst, src)
                ncop[0] += 1

            with (
                tc.tile_pool(name="work", bufs=6, space="PSUM") as work,
                tc.tile_pool(name="psl", bufs=1, space="PSUM") as psl,
            ):
                lg_a = psl.tile([128, C], f32, tag="lga")
                lg_b = psl.tile([64, C], f32, tag="lgb")

                # pre-warm the PE HAM clock during the startup DMA wait;
                # keyed off the x DMA, which lands before the weights
                junk0 = work.tile([128, 512], f32, tag="w")
                for _ in range(8):
                    nc.tensor.matmul(junk0[:, :], xa0[:, 1, 0:128],
                                     xa0[:, 1:5, :],
                                     start=True, stop=True, skip_group_check=True)

                pending_squares = None
                for ch in range(NCHUNK):
                    y0 = ch * TY
                    # ---- load x chunk (rows y0-1 .. y0+16) ----
                    if ch == 0:
                        xa, xb = xa0, xb0
                    else:
                        xa = xpool.tile([128, SLOTS, W], bf16, tag="xa")
                        xb = xb0 if ch % 2 == 0 else xb1
                        lo = y0 - 1
                        hi = min(y0 + TY, H - 1)
                        s1 = hi - (y0 - 1)
                        if ch == NCHUNK - 1:
                            nc.vector.memset(xa[:, SLOTS - 1, :], 0.0)
                            nc.vector.memset(xb[0:64, SLOTS - 1, :], 0.0)
                        nc.sync.dma_start(xa[:, 0:s1 + 1, :], x_d[0:128, lo:hi + 1, :])
                        nc.sync.dma_start(xb[0:64, 0:s1 + 1, :],
                                          x_d[128:192, lo:hi + 1, :])

                    # ---- GEMM1: qkv = wq.T @ x, 3 rows (384 cols) at a time ----
                    for r in range(5):
                        po, pn = 128 * r, PT5[r]
                        for j in range(SLOTS // 3):
                            pg = work.tile([128, 3, W], f32, tag="w")
                            nc.tensor.matmul(
                                pg[:, :, :], wq_a[:, po:po + 128],
                                xa[:, 3 * j:3 * j + 3, :], start=True, stop=False)
                            nc.tensor.matmul(
                                pg[:, :, :], wq_b[:, po:po + 128],
                                xb[:, 3 * j:3 * j + 3, :], start=False, stop=True)
                            evac(qkv[0:pn, r, 3 * j:3 * j + 3, 1:1 + W],
                                 pg[0:pn, :, :])

                    # ---- previous chunk's L2-norm squares run in ACT's slack ----
                    if pending_squares is not None:
                        pending_squares()
                        pending_squares = None

                    # ---- depthwise 3x3 for q,k tiles (r = 0, 1, 3) ----
                    q_a = qkpool.tile([128, TY, W], bf16, tag="qa")
                    k_a = qkpool.tile([128, TY, W], bf16, tag="ka")
                    qk_b = qkpool.tile([128, TY, W], bf16, tag="qkb")
                    dstmap = {0: q_a, 1: k_a, 3: qk_b}
                    for r in (0, 1, 3):
                        for t in range(TY // 4):
                            pd = work.tile([128, 4, W], f32, tag="w")
                            for kk in range(9):
                                dy, dx = kk // 3 - 1, kk % 3 - 1
                                srow = 1 + 4 * t + dy
                                nc.tensor.matmul(
                                    pd[:, :, :],
                                    wdg[:, r * 9 + kk, :],
                                    qkv[:, r, srow:srow + 4, 1 + dx:1 + dx + W],
                                    start=(kk == 0), stop=(kk == 8))
                            d = dstmap[r]
                            evac(d[:, 4 * t:4 * t + 4, :], pd[:, :, :])

                    # ---- L2 norm partial sums (ACT: square + accum), deferred
                    #      one chunk so GEMM1's evac copies aren't queued
                    #      behind them on ACT ----
                    def make_squares(q_a=q_a, k_a=k_a, qk_b=qk_b, ch=ch):
                        def emit():
                            nc.scalar.activation(sqs[:, :, :], q_a[:, :, :],
                                                 Act.Square,
                                                 accum_out=ssA[:, 0, ch:ch + 1])
                            nc.scalar.activation(sqs[0:64, :, :], qk_b[0:64, :, :],
                                                 Act.Square,
                                                 accum_out=ssB[0:64, ch:ch + 1])
                            nc.scalar.activation(sqs[:, :, :], k_a[:, :, :],
                                                 Act.Square,
                                                 accum_out=ssA[:, 1, ch:ch + 1])
                            nc.scalar.activation(sqs[64:128, :, :],
                                                 qk_b[64:128, :, :], Act.Square,
                                                 accum_out=ssB[64:128, ch:ch + 1])
                        return emit
                    if ch == NCHUNK - 1:
                        # last chunk: run now so the sums are ready when the
                        # softmax chain starts (overlaps dw-v / logits)
                        make_squares()()
                        pending_squares = None
                    else:
                        pending_squares = make_squares()

                    # ---- chunk-wise q/k transpose on the DMA xbar ----
                    nc.sync.dma_start(qT[:, :, 0:128], q_a[:, :, :], transpose=True)
                    nc.sync.dma_start(qT[:, :, 128:192], qk_b[0:64, :, :], transpose=True)
                    nc.sync.dma_start(kT[:, :, 0:128], k_a[:, :, :], transpose=True)
                    nc.sync.dma_start(kT[:, :, 128:192], qk_b[64:128, :, :], transpose=True)

                    # ---- depthwise for v tiles (r = 2, 4); overlaps the xbar ----
                    for r in (2, 4):
                        for t in range(TY // 4):
                            pd = work.tile([128, 4, W], f32, tag="w")
                            for kk in range(9):
                                dy, dx = kk // 3 - 1, kk % 3 - 1
                                srow = 1 + 4 * t + dy
                                nc.tensor.matmul(
                                    pd[:, :, :],
                                    wdg[:, r * 9 + kk, :],
                                    qkv[:, r, srow:srow + 4, 1 + dx:1 + dx + W],
                                    start=(kk == 0), stop=(kk == 8))
                            dst = y0 + 4 * t
                            if r == 2:
                                evac(v_a[:, dst:dst + 4, :], pd[:, :, :])
                            else:
                                evac(v_b[0:64, dst:dst + 4, :], pd[0:64, :, :])

                    # ---- accumulate logits (grouped by M so the PE array
                    #      geometry stays constant within each run) ----
                    for s in range(TY):
                        first = (ch == 0 and s == 0)
                        last = (ch == NCHUNK - 1 and s == TY - 1)
                        nc.tensor.matmul(lg_a[:, :], qT[:, s, 0:128], kT[:, s, :],
                                         start=first, stop=last, skip_group_check=True)
                    for s in range(TY):
                        first = (ch == 0 and s == 0)
                        last = (ch == NCHUNK - 1 and s == TY - 1)
                        nc.tensor.matmul(lg_b[:, :], qT[:, s, 128:192], kT[:, s, :],
                                         start=first, stop=last, skip_group_check=True)

                if pending_squares is not None:
                    pending_squares()
                    pending_squares = None

                # PE warm-keeper: junk matmuls with staged deps on the softmax
                # chain so the HAM clock never sees a ~3.4us idle window.
                junk = work.tile([128, 512], f32, tag="w")
                sqs_f32 = sqs[:, :, :].bitcast(f32)   # junk rhs, N=512

                def filler(dep):
                    m = dep.free_size()
                    nc.tensor.matmul(junk[0:m, :], dep, sqs_f32[:, 0:8, :],
                                     start=True, stop=True, skip_group_check=True)

                # ================= softmax prep =================
                # rrA[:, 0] = rq (q lo/tv-scaled), rrA[:, 1] = rk (k lo);
                # rrB[0:64] = rq hi, rrB[64:128] = rk hi
                rrA = small.tile([128, 2], f32, tag="rrA")
                rrB = small.tile([128, 1], f32, tag="rrB")
                tmpA = small.tile([128, 2], f32, tag="tmpA")
                tmpB = small.tile([128, 1], f32, tag="tmpB")
                nc.vector.tensor_reduce(tmpA[:], ssA[:, :, :],
                                        mybir.AxisListType.X, Alu.add)
                nc.vector.tensor_reduce(tmpB[:], ssB[:, :],
                                        mybir.AxisListType.X, Alu.add)
                nc.scalar.activation(tmpA[:], tmpA[:], Act.Sqrt)
                nc.scalar.activation(tmpB[:], tmpB[:], Act.Sqrt)
                nc.vector.tensor_scalar_max(tmpA[:], tmpA[:], 1e-12)
                nc.vector.tensor_scalar_max(tmpB[:], tmpB[:], 1e-12)
                nc.vector.reciprocal(rrA[:], tmpA[:])
                nc.vector.reciprocal(rrB[:], tmpB[:])
                filler(tmpA[:, :])
                nc.vector.tensor_tensor(rrA[:, 0:1], rrA[:, 0:1], tv_a[:], Alu.mult)
                nc.vector.tensor_tensor(rrB[0:64, :], rrB[0:64, :], tv_b[:], Alu.mult)
                filler(rrB[:, :])
                rq_a, rk_a = rrA[:, 0:1], rrA[:, 1:2]
                rq_b, rk_b = rrB[0:64, :], rrB[64:128, :]

                # copy logits out of psum, scale rows by rq
                L_a = small.tile([128, C], f32, tag="La")
                L_b = small.tile([64, C], f32, tag="Lb")
                nc.vector.tensor_scalar(L_a[:], lg_a[:], rq_a, None, Alu.mult)
                nc.vector.tensor_scalar(L_b[:], lg_b[:], rq_b, None, Alu.mult)
                filler(L_a[:, 0:128])

            with tc.tile_pool(name="psx", bufs=1, space="PSUM") as psx:
                junk2 = psx.tile([128, 512], f32, tag="junk2")

                def filler2(dep):
                    m = dep.free_size()
                    nc.tensor.matmul(junk2[0:m, :], dep, sqs_f32[:, 0:8, :],
                                     start=True, stop=True, skip_group_check=True)

                # column scale: bcast rk over partitions via K=1 matmul
                rkrow = small.tile([1, C], f32, tag="rkrow")
                pb = psx.tile([128, C], f32, tag="pb")
                nc.tensor.transpose(pb[0:1, 0:128], rk_a, idf[:])
                nc.tensor.transpose(pb[0:1, 128:192], rk_b,
                                    idf[64:128, 64:128])
                nc.any.tensor_copy(rkrow[:], pb[0:1, 0:192])
                pbc = psx.tile([128, C], f32, tag="pbc")
                nc.tensor.matmul(pbc[:, :], ones1[:], rkrow[:], start=True, stop=True)
                nc.vector.tensor_tensor(L_a[:], L_a[:], pbc[:, :], Alu.mult)
                nc.vector.tensor_tensor(L_b[:], L_b[:], pbc[0:64, :], Alu.mult)
                filler2(L_a[:, 0:128])

                # full-row softmax; cross-head blocks masked to -1e30 -> exp 0.
                # a/b halves use independent scratch so their DVE/ACT ops
                # interleave instead of serializing.
                mx = small.tile([128, 1], f32, tag="mx")
                sm = small.tile([128, 1], f32, tag="sm")
                E = small.tile([128, C], f32, tag="E")
                mx2 = small.tile([64, 1], f32, tag="mx2")
                sm2 = small.tile([64, 1], f32, tag="sm2")
                E2 = small.tile([64, C], f32, tag="E2")
                nc.vector.tensor_tensor(L_a[:], L_a[:], mk_a[:], Alu.add)
                nc.vector.tensor_tensor(L_b[:], L_b[:], mk_b[:], Alu.add)
                nc.vector.tensor_reduce(mx[:], L_a[:], mybir.AxisListType.X, Alu.max)
                nc.vector.tensor_reduce(mx2[:], L_b[:], mybir.AxisListType.X, Alu.max)
                nc.vector.tensor_scalar_mul(mx[:], mx[:], -1.0)
                nc.vector.tensor_scalar_mul(mx2[:], mx2[:], -1.0)
                filler2(mx[:, :])
                nc.scalar.activation(E[:, :], L_a[:], Act.Exp,
                                     bias=mx[:], accum_out=sm[:])
                nc.scalar.activation(E2[:, :], L_b[:], Act.Exp,
                                     bias=mx2[:], accum_out=sm2[:])
                nc.vector.reciprocal(sm[:], sm[:])
                nc.vector.reciprocal(sm2[:], sm2[:])
                filler2(E[:, 0:128])
                nc.scalar.activation(attn_a[:, 0:C], E[:, :], Act.Copy,
                                     scale=sm[:])
                nc.scalar.activation(attn_b[0:64, 0:C], E2[:, :], Act.Copy,
                                     scale=sm2[:])
                filler2(E[:, 0:128])

                # fold projection into attention: FT[d, o] = sum_c attn[c,d] wp[c,o]
                pfa = psx.tile([128, C], f32, tag="pfa")
                pfb = psx.tile([128, C], f32, tag="pfb")
                nc.tensor.matmul(pfa[:, :], attn_a[:, 0:128], wp_a[:, :],
                                 start=True, stop=False)
                nc.tensor.matmul(pfa[:, :], attn_b[:, 0:128], wp_b[:, :],
                                 start=False, stop=True)
                nc.tensor.matmul(pfb[:, :], attn_a[:, 128:256], wp_a[:, :],
                                 start=True, stop=False)
                nc.tensor.matmul(pfb[:, :], attn_b[:, 128:256], wp_b[:, :],
                                 start=False, stop=True)
                nc.vector.tensor_copy(FT_a[:, 0:C], pfa[:, :])
                nc.scalar.copy(FT_b[0:64, 0:C], pfb[0:64, :])

            # ---- pass 2: out = FT.T @ v, streamed to HBM in bf16.
            #      Output staged in 8-row tiles so each DMA moves 256 KB. ----
            TB = 8
            with (
                tc.tile_pool(name="pout", bufs=4, space="PSUM") as pout,
                tc.tile_pool(name="o2", bufs=3) as opool,
            ):
                for blk in range(H // TB):
                    yb = blk * TB
                    ot_a = opool.tile([128, TB, W], bf16, tag="ota")
                    ot_b = opool.tile([64, TB, W], bf16, tag="otb")
                    for t in range(TB // 4):
                        r4 = yb + 4 * t
                        poa = pout.tile([128, 4, W], f32, tag="poa")
                        pob = pout.tile([128, 4, W], f32, tag="pob")
                        nc.tensor.matmul(poa[:, :, :], FT_a[:, 0:128],
                                         v_a[:, r4:r4 + 4, :], start=True, stop=False)
                        nc.tensor.matmul(poa[:, :, :], FT_b[:, 0:128],
                                         v_b[:, r4:r4 + 4, :], start=False, stop=True)
                        nc.tensor.matmul(pob[:, :, :], FT_a[:, 128:256],
                                         v_a[:, r4:r4 + 4, :], start=True, stop=False)
                        nc.tensor.matmul(pob[:, :, :], FT_b[:, 128:256],
                                         v_b[:, r4:r4 + 4, :], start=False, stop=True)
                        evac(ot_a[:, 4 * t:4 * t + 4, :], poa[:, :, :])
                        evac(ot_b[:, 4 * t:4 * t + 4, :], pob[0:64, :, :])
                    nc.sync.dma_start(out_d[0:128, yb:yb + TB, :], ot_a[:])
                    nc.sync.dma_start(out_d[128:192, yb:yb + TB, :], ot_b[:])

    nc.compile()
    return nc


# permuted channel order: tile0=q[0:128], tile1=k[0:128], tile2=v[0:128],
# tile3=q[128:192]+k[128:192], tile4=v[128:192]  (orig rows of w_qkv)
def _perm():
    return np.concatenate([
        np.arange(0, 128),          # q lo
        np.arange(192, 320),        # k lo
        np.arange(384, 512),        # v lo
        np.arange(128, 192),        # q hi
        np.arange(320, 384),        # k hi
        np.arange(512, 576),        # v hi
    ])


def _prep_weights(w_qkv, w_dw, w_project, temperature):
    perm = _perm()
    wqt = w_qkv[perm].T                                          # [192, 576]
    # pad to [128, 640]: wqa = input ch 0:128; wqb = input ch 128:192 + zeros
    wqa = np.zeros((128, C3P), np.float32)
    wqb = np.zeros((128, C3P), np.float32)
    for r in range(5):
        pn = PT5[r]
        wqa[:, 128 * r:128 * r + pn] = wqt[0:128, PO5[r]:PO5[r] + pn]
        wqb[0:64, 128 * r:128 * r + pn] = wqt[128:192, PO5[r]:PO5[r] + pn]
    # diagonal stationaries: block (r*9+k) = diag(w_dw[perm ch, k])
    wdg = np.zeros((128, 45, 128), np.float32)
    wd = w_dw.reshape(C3, 9)[perm]
    for r in range(5):
        po, pn = PO5[r], PT5[r]
        for k in range(9):
            blk = wdg[:, r * 9 + k, :]
            blk[np.arange(pn), np.arange(pn)] = wd[po:po + pn, k]
    wpt = w_project.T                                            # [c, o]
    wpa = wpt[0:128]
    wpb = np.zeros((128, C), np.float32)
    wpb[0:64] = wpt[128:192]
    tv = np.repeat(temperature.reshape(HEADS), DH).reshape(C, 1).astype(np.float32)
    mk = np.full((C, C), -1e30, np.float32)
    for h in range(HEADS):
        mk[h * DH:(h + 1) * DH, h * DH:(h + 1) * DH] = 0.0
    return {
        "wqa": wqa.astype(BF16),
        "wqb": wqb.astype(BF16),
        "wpa": np.ascontiguousarray(wpa).astype(BF16),
        "wpb": wpb.astype(BF16),
        "wdg": wdg.reshape(128, 45 * 128).astype(BF16),
        "tv": tv,
        "idf": np.eye(128, dtype=np.float32),
        "ones1": np.ones((1, 128), np.float32),
        "mka": mk[0:128],
        "mkb": mk[128:192],
    }


def kernel(x, w_qkv, w_dw, w_project, temperature, heads):
    from concourse import bass_utils

    x = np.asarray(x, np.float32)
    key = "nc"
    if key not in _CACHE:
        _CACHE[key] = _build()
    nc = _CACHE[key]

    shared = _prep_weights(np.asarray(w_qkv, np.float32),
                           np.asarray(w_dw, np.float32),
                           np.asarray(w_project, np.float32),
                           np.asarray(temperature, np.float32))
    in_maps = []
    for i in range(B):
        m = dict(shared)
        m["x0"] = x[i].reshape(C, H, W).astype(BF16)
        in_maps.append(m)

    res = bass_utils.run_bass_kernel_spmd(nc, in_maps, core_ids=list(range(B)))
    outs = [r["out0"].reshape(C, H, W) for r in res.results]
    return np.stack(outs, axis=0).astype(np.float32)


if __name__ == "__main__":
    rng = np.random.default_rng(0)
    x = rng.standard_normal((B, C, H, W)).astype(np.float32)
    w_qkv = (rng.standard_normal((C3, C)) / np.sqrt(C)).astype(np.float32)
    w_dw = (rng.standard_normal((C3, 1, 3, 3)) / 3.0).astype(np.float32)
    w_project = (rng.standard_normal((C, C)) / np.sqrt(C)).astype(np.float32)
    temperature = np.ones((HEADS, 1, 1), np.float32)
    y = kernel(x=x, w_qkv=w_qkv, w_dw=w_dw, w_project=w_project,
               temperature=temperature, heads=HEADS)
    print(y.shape, y.dtype)


# revision 30
# speedup vs baseline: 1.0130x; 1.0018x over previous
"""MDTA (Restormer transposed-channel attention) Trainium2 Bass kernel.

Data-parallel over batch: 8 images -> 8 NeuronCores, one image per core.

Per-core pipeline (image = [192, 128, 128] f32):
  1. GEMM1 (1x1 conv): qkv = w_qkv @ x on PE, bf16 in / f32 psum, written
     into a row-padded buffer (row stride 130) so the depthwise conv reads
     zeros at image edges. Channels permuted into 5 partition tiles:
     [q0:128, k0:128, v0:128, q128:192|k128:192, v128:192].
  2. Depthwise 3x3: 9 accumulating PE matmuls per 4-row tile with diagonal
     stationaries; tap shift via rhs AP offset into the padded buffer.
  3. q/k transposed chunk-wise with 4 DMA-transpose (xbar) instructions;
     logits qT.T@kT accumulate into persistent PSUM; L2-norm sums via ACT
     Square+accum_out. v's depthwise runs after the transposes are issued
     so PE stays busy while the xbar works.
  4. Softmax on rsqrt-scaled logits (block-diag mask per head), then the
     projection is folded into attention: FT = (attn.T @ wp) so that
     out = FT.T @ v is a single GEMM pass streamed straight to HBM (bf16).

All matmuls are zero-padded to K=128/M=128 (padding rows/cols cost nothing:
stream time depends only on N) because the PE only pipelines LDWEIGHTS under
the previous matmul when consecutive matmuls keep the same array geometry.
Filler matmuls with staged dependencies keep the PE HAM clock warm through
the softmax phase so pass 2 runs at 2.4 GHz instead of 1.2 GHz.
"""

import os
import sys
import numpy as np

for _p in ("/opt/trn_rl_repo",):
    if _p not in sys.path and os.path.isdir(_p):
        sys.path.insert(0, _p)

import ml_dtypes

BF16 = ml_dtypes.bfloat16

B, C, H, W = 8, 192, 128, 128
HEADS, DH = 4, 48
C3 = 3 * C            # 576
C3P = 640             # padded (5 x 128 output-channel tiles)
HW = H * W            # 16384
RT = W + 2            # padded row stride = 130
TY = 16               # image rows per chunk
NCHUNK = H // TY      # 8
SLOTS = TY + 2        # 18 row slots per chunk (halo)
# partition tiles over the PERMUTED 576 channels
PT5 = [128, 128, 128, 128, 64]
PO5 = [0, 128, 256, 384, 512]

_CACHE = {}


def _build():
    import concourse.bass as bass
    import concourse.bacc as bacc
    import concourse.tile as tile
    import concourse.mybir as mybir

    f32 = mybir.dt.float32
    bf16 = mybir.dt.bfloat16
    Alu = mybir.AluOpType
    Act = mybir.ActivationFunctionType

    nc = bacc.Bacc("TRN2", target_bir_lowering=False, debug=False,
                   enable_asserts=False)

    x_d = nc.dram_tensor("x0", [C, H, W], bf16, kind="ExternalInput").ap()
    wqa_d = nc.dram_tensor("wqa", [128, C3P], bf16, kind="ExternalInput").ap()
    wqb_d = nc.dram_tensor("wqb", [128, C3P], bf16, kind="ExternalInput").ap()
    wdg_d = nc.dram_tensor("wdg", [128, 45 * 128], bf16, kind="ExternalInput").ap()
    wpa_d = nc.dram_tensor("wpa", [128, C], bf16, kind="ExternalInput").ap()
    wpb_d = nc.dram_tensor("wpb", [128, C], bf16, kind="ExternalInput").ap()
    tv_d = nc.dram_tensor("tv", [C, 1], f32, kind="ExternalInput").ap()
    idf_d = nc.dram_tensor("idf", [128, 128], f32, kind="ExternalInput").ap()
    ones_d = nc.dram_tensor("ones1", [1, 128], f32, kind="ExternalInput").ap()
    mka_d = nc.dram_tensor("mka", [128, C], f32, kind="ExternalInput").ap()
    mkb_d = nc.dram_tensor("mkb", [64, C], f32, kind="ExternalInput").ap()
    out_d = nc.dram_tensor("out0", [C, H, W], bf16, kind="ExternalOutput").ap()

    with tile.TileContext(nc) as tc:
        with (
            tc.tile_pool(name="const", bufs=1) as cpool,
            tc.tile_pool(name="big", bufs=1) as big,
            tc.tile_pool(name="xin", bufs=2) as xpool,
            tc.tile_pool(name="qk", bufs=2) as qkpool,
            tc.tile_pool(name="qt", bufs=1) as qtpool,
            tc.tile_pool(name="small", bufs=1) as small,
        ):
            # ---- persistent big buffers ----
            qkv = big.tile([128, 5, SLOTS, RT], bf16, tag="qkv")
            v_a = big.tile([128, H, W], bf16, tag="va")
            v_b = big.tile([128, H, W], bf16, tag="vb")   # rows 64:128 stay 0

            wq_a = cpool.tile([128, C3P], bf16, tag="wqa")
            wq_b = cpool.tile([128, C3P], bf16, tag="wqb")  # rows 64:128 zero
            wdg = cpool.tile([128, 45, 128], bf16, tag="wdg")
            wp_a = cpool.tile([128, C], bf16, tag="wpa")
            wp_b = cpool.tile([128, C], bf16, tag="wpb")    # rows 64:128 zero
            idf = cpool.tile([128, 128], f32, tag="idf")
            ones1 = cpool.tile([1, 128], f32, tag="ones1")
            tv_a = cpool.tile([128, 1], f32, tag="tva")
            tv_b = cpool.tile([64, 1], f32, tag="tvb")
            mk_a = cpool.tile([128, C], f32, tag="mka")
            mk_b = cpool.tile([64, C], f32, tag="mkb")
            # explicit double buffer for xb so the zero padding (rows 64:128)
            # survives across chunks
            xb0 = cpool.tile([128, SLOTS, W], bf16, tag="xb0")
            xb1 = cpool.tile([128, SLOTS, W], bf16, tag="xb1")

            # first x chunk starts the critical path on the SP HWDGE ring;
            # weights go down the ACT HWDGE ring in parallel
            xa0 = xpool.tile([128, SLOTS, W], bf16, tag="xa")
            nc.vector.memset(xa0[:, 0, :], 0.0)
            nc.vector.memset(xb0[0:64, 0, :], 0.0)
            # split the startup transfers so chunk-0 GEMM1 starts on the
            # first pieces while the rest stream in
            nc.sync.dma_start(xa0[:, 1:10, :], x_d[0:128, 0:9, :])
            nc.sync.dma_start(xb0[0:64, 1:10, :], x_d[128:192, 0:9, :])
            nc.sync.dma_start(wq_a[:, 0:128], wqa_d[:, 0:128])
            nc.sync.dma_start(wq_b[:, 0:128], wqb_d[:, 0:128])
            nc.sync.dma_start(xa0[:, 10:SLOTS, :], x_d[0:128, 9:SLOTS - 1, :])
            nc.sync.dma_start(xb0[0:64, 10:SLOTS, :], x_d[128:192, 9:SLOTS - 1, :])
            nc.sync.dma_start(wq_a[:, 128:C3P], wqa_d[:, 128:C3P])
            nc.sync.dma_start(wq_b[:, 128:C3P], wqb_d[:, 128:C3P])
            nc.sync.dma_start(wdg[:], wdg_d[:, :])
            nc.scalar.dma_start(wp_a[:], wpa_d[:])
            nc.scalar.dma_start(wp_b[:], wpb_d[:])
            nc.scalar.dma_start(idf[:], idf_d[:])
            nc.scalar.dma_start(ones1[:], ones_d[:])
            nc.scalar.dma_start(tv_a[:], tv_d[0:128, :])
            nc.scalar.dma_start(tv_b[:], tv_d[128:192, :])
            nc.scalar.dma_start(mk_a[:], mka_d[:])
            nc.scalar.dma_start(mk_b[:], mkb_d[:])

            # only the pad columns / pad partitions of qkv need zeroing
            # (GEMM1 rewrites every data column each chunk)
            nc.vector.memset(qkv[:, :, :, 0:RT:RT - 1], 0.0)
            # zero pads on GpSimd (otherwise idle); xb pads first — chunk-0
            # GEMM1 depends on them. v_b's big pad isn't read until pass 2.
            nc.gpsimd.memset(xb0[64:128, :, :], 0.0)
            nc.gpsimd.memset(xb1[64:128, :, :], 0.0)
            nc.gpsimd.memset(qkv[64:128, 4, :, :], 0.0)
            nc.gpsimd.memset(v_b[64:128, :, :], 0.0)

            # l2 norm partial sums per chunk, packed for a short rsqrt chain:
            # ssA free-dim-packs q_a/k_a sums; ssB partition-packs q_b (0:64)
            # and k_b (64:128) sums
            ssA = small.tile([128, 2, NCHUNK], f32, tag="ssA")
            ssB = small.tile([128, NCHUNK], f32, tag="ssB")

            sqs = big.tile([128, TY, W], bf16, tag="sqs")
            qT = qtpool.tile([128, TY, C], bf16, tag="qT")
            kT = qtpool.tile([128, TY, C], bf16, tag="kT")

            # padded tail tensors (pads zeroed once, on GpSimd)
            attn_a = small.tile([128, 256], bf16, tag="atta")
            attn_b = small.tile([128, 256], bf16, tag="attb")
            FT_a = small.tile([128, 256], bf16, tag="FTa")
            FT_b = small.tile([128, 256], bf16, tag="FTb")
            nc.gpsimd.memset(attn_a[:, C:256], 0.0)
            nc.gpsimd.memset(attn_b[:], 0.0)
            nc.gpsimd.memset(FT_a[:], 0.0)
            nc.gpsimd.memset(FT_b[:], 0.0)

            ncop = [0]

            def evac(dst, src):
                # alternate PSUM evacuation between DVE and ACT
                if ncop[0] % 2 == 0:
                    nc.vector.tensor_copy(dst, src)
                else:
                    nc.scalar.copy(dst, src)
                ncop[0] += 1

            with (
                tc.tile_pool(name="work", bufs=6, space="PSUM") as work,
                tc.tile_pool(name="psl", bufs=1, space="PSUM") as psl,
            ):
                lg_a = psl.tile([128, C], f32, tag="lga")
                lg_b = psl.tile([64, C], f32, tag="lgb")

                # pre-warm the PE HAM clock during the startup DMA wait;
                # keyed off the x DMA, which lands before the weights
                junk0 = work.tile([128, 512], f32, tag="w")
                for _ in range(8):
                    nc.tensor.matmul(junk0[:, :], xa0[:, 1, 0:128],
                                     xa0[:, 1:5, :],
                                     start=True, stop=True, skip_group_check=True)

                pending_squares = None
                for ch in range(NCHUNK):
                    y0 = ch * TY
                    # ---- load x chunk (rows y0-1 .. y0+16) ----
                    if ch == 0:
                        xa, xb = xa0, xb0
                    else:
                        xa = xpool.tile([128, SLOTS, W], bf16, tag="xa")
                        xb = xb0 if ch % 2 == 0 else xb1
                        lo = y0 - 1
                        hi = min(y0 + TY, H - 1)
                        s1 = hi - (y0 - 1)
                        if ch == NCHUNK - 1:
                            nc.vector.memset(xa[:, SLOTS - 1, :], 0.0)
                            nc.vector.memset(xb[0:64, SLOTS - 1, :], 0.0)
                        nc.sync.dma_start(xa[:, 0:s1 + 1, :], x_d[0:128, lo:hi + 1, :])
                        nc.sync.dma_start(xb[0:64, 0:s1 + 1, :],
                                          x_d[128:192, lo:hi + 1, :])

                    # ---- GEMM1: qkv = wq.T @ x, 3 rows (384 cols) at a time ----
                    for r in range(5):
                        po, pn = 128 * r, PT5[r]
                        for j in range(SLOTS // 3):
                            pg = work.tile([128, 3, W], f32, tag="w")
                            nc.tensor.matmul(
                                pg[:, :, :], wq_a[:, po:po + 128],
                                xa[:, 3 * j:3 * j + 3, :], start=True, stop=False)
                            nc.tensor.matmul(
                                pg[:, :, :], wq_b[:, po:po + 128],
                                xb[:, 3 * j:3 * j + 3, :], start=False, stop=True)
                            evac(qkv[0:pn, r, 3 * j:3 * j + 3, 1:1 + W],
                                 pg[0:pn, :, :])

                    # ---- previous chunk's L2-norm squares run in ACT's slack ----
                    if pending_squares is not None:
                        pending_squares()
                        pending_squares = None

                    # ---- depthwise 3x3 for q,k tiles (r = 0, 1, 3) ----
                    q_a = qkpool.tile([128, TY, W], bf16, tag="qa")
                    k_a = qkpool.tile([128, TY, W], bf16, tag="ka")
                    qk_b = qkpool.tile([128, TY, W], bf16, tag="qkb")
                    dstmap = {0: q_a, 1: k_a, 3: qk_b}
                    for r in (0, 1, 3):
                        for t in range(TY // 4):
                            pd = work.tile([128, 4, W], f32, tag="w")
                            for kk in range(9):
                                dy, dx = kk // 3 - 1, kk % 3 - 1
                                srow = 1 + 4 * t + dy
                                nc.tensor.matmul(
                                    pd[:, :, :],
                                    wdg[:, r * 9 + kk, :],
                                    qkv[:, r, srow:srow + 4, 1 + dx:1 + dx + W],
                                    start=(kk == 0), stop=(kk == 8))
                            d = dstmap[r]
                            evac(d[:, 4 * t:4 * t + 4, :], pd[:, :, :])

                    # ---- L2 norm partial sums (ACT: square + accum), deferred
                    #      one chunk so GEMM1's evac copies aren't queued
                    #      behind them on ACT ----
                    def make_squares(q_a=q_a, k_a=k_a, qk_b=qk_b, ch=ch):
                        def emit():
                            nc.scalar.activation(sqs[:, :, :], q_a[:, :, :],
                                                 Act.Square,
                                                 accum_out=ssA[:, 0, ch:ch + 1])
                            nc.scalar.activation(sqs[0:64, :, :], qk_b[0:64, :, :],
                                                 Act.Square,
                                                 accum_out=ssB[0:64, ch:ch + 1])
                            nc.scalar.activation(sqs[:, :, :], k_a[:, :, :],
                                                 Act.Square,
                                                 accum_out=ssA[:, 1, ch:ch + 1])
                            nc.scalar.activation(sqs[64:128, :, :],
                                                 qk_b[64:128, :, :], Act.Square,
                                                 accum_out=ssB[64:128, ch:ch + 1])
                        return emit
                    if ch == NCHUNK - 1:
                        # last chunk: run now so the sums are ready when the
                        # softmax chain starts (overlaps dw-v / logits)
                        make_squares()()
                        pending_squares = None
                    else:
                        pending_squares = make_squares()

                    # ---- chunk-wise q/k transpose on the DMA xbar ----
                    nc.sync.dma_start(qT[:, :, 0:128], q_a[:, :, :], transpose=True)
                    nc.sync.dma_start(qT[:, :, 128:192], qk_b[0:64, :, :], transpose=True)
                    nc.sync.dma_start(kT[:, :, 0:128], k_a[:, :, :], transpose=True)
                    nc.sync.dma_start(kT[:, :, 128:192], qk_b[64:128, :, :], transpose=True)

                    # ---- depthwise for v tiles (r = 2, 4); overlaps the xbar ----
                    for r in (2, 4):
                        for t in range(TY // 4):
                            pd = work.tile([128, 4, W], f32, tag="w")
                            for kk in range(9):
                                dy, dx = kk // 3 - 1, kk % 3 - 1
                                srow = 1 + 4 * t + dy
                                nc.tensor.matmul(
                                    pd[:, :, :],
                                    wdg[:, r * 9 + kk, :],
                                    qkv[:, r, srow:srow + 4, 1 + dx:1 + dx + W],
                                    start=(kk == 0), stop=(kk == 8))
                            dst = y0 + 4 * t
                            if r == 2:
                                evac(v_a[:, dst:dst + 4, :], pd[:, :, :])
                            else:
                                evac(v_b[0:64, dst:dst + 4, :], pd[0:64, :, :])

                    # ---- accumulate logits (grouped by M so the PE array
                    #      geometry stays constant within each run) ----
                    for s in range(TY):
                        first = (ch == 0 and s == 0)
                        last = (ch == NCHUNK - 1 and s == TY - 1)
                        nc.tensor.matmul(lg_a[:, :], qT[:, s, 0:128], kT[:, s, :],
                                         start=first, stop=last, skip_group_check=True)
                    for s in range(TY):
                        first = (ch == 0 and s == 0)
                        last = (ch == NCHUNK - 1 and s == TY - 1)
                        nc.tensor.matmul(lg_b[:, :], qT[:, s, 128:192], kT[:, s, :],
                                         start=first, stop=last, skip_group_check=True)

                if pending_squares is not None:
                    pending_squares()
                    pending_squares = None

                # PE warm-keeper: junk matmuls with staged deps on the softmax
                # chain so the HAM clock never sees a ~3.4us idle window.
                junk = work.tile([128, 512], f32, tag="w")
                sqs_f32 = sqs[:, :, :].bitcast(f32)   # junk rhs, N=512

                def filler(dep):
                    m = dep.free_size()
                    nc.tensor.matmul(junk[0:m, :], dep, sqs_f32[:, 0:8, :],
                                     start=True, stop=True, skip_group_check=True)

                # ================= softmax prep =================
                # rrA[:, 0] = rq (q lo/tv-scaled), rrA[:, 1] = rk (k lo);
                # rrB[0:64] = rq hi, rrB[64:128] = rk hi
                rrA = small.tile([128, 2], f32, tag="rrA")
                rrB = small.tile([128, 1], f32, tag="rrB")
                tmpA = small.tile([128, 2], f32, tag="tmpA")
                tmpB = small.tile([128, 1], f32, tag="tmpB")
                nc.vector.tensor_reduce(tmpA[:], ssA[:, :, :],
                                        mybir.AxisListType.X, Alu.add)
                nc.vector.tensor_reduce(tmpB[:], ssB[:, :],
                                        mybir.AxisListType.X, Alu.add)
                nc.scalar.activation(tmpA[:], tmpA[:], Act.Sqrt)
                nc.scalar.activation(tmpB[:], tmpB[:], Act.Sqrt)
                nc.vector.tensor_scalar_max(tmpA[:], tmpA[:], 1e-12)
                nc.vector.tensor_scalar_max(tmpB[:], tmpB[:], 1e-12)
                nc.vector.reciprocal(rrA[:], tmpA[:])
                nc.vector.reciprocal(rrB[:], tmpB[:])
                filler(tmpA[:, :])
                nc.vector.tensor_tensor(rrA[:, 0:1], rrA[:, 0:1], tv_a[:], Alu.mult)
                nc.vector.tensor_tensor(rrB[0:64, :], rrB[0:64, :], tv_b[:], Alu.mult)
                filler(rrB[:, :])
                rq_a, rk_a = rrA[:, 0:1], rrA[:, 1:2]
                rq_b, rk_b = rrB[0:64, :], rrB[64:128, :]

                # copy logits out of psum, scale rows by rq
                L_a = small.tile([128, C], f32, tag="La")
                L_b = small.tile([64, C], f32, tag="Lb")
                nc.vector.tensor_scalar(L_a[:], lg_a[:], rq_a, None, Alu.mult)
                nc.vector.tensor_scalar(L_b[:], lg_b[:], rq_b, None, Alu.mult)
                filler(L_a[:, 0:128])

            with tc.tile_pool(name="psx", bufs=1, space="PSUM") as psx:
                junk2 = psx.tile([128, 512], f32, tag="junk2")

                def filler2(dep):
                    m = dep.free_size()
                    nc.tensor.matmul(junk2[0:m, :], dep, sqs_f32[:, 0:8, :],
                                     start=True, stop=True, skip_group_check=True)

                # column scale: bcast rk over partitions via K=1 matmul
                rkrow = small.tile([1, C], f32, tag="rkrow")
                pb = psx.tile([128, C], f32, tag="pb")
                nc.tensor.transpose(pb[0:1, 0:128], rk_a, idf[:])
                nc.tensor.transpose(pb[0:1, 128:192], rk_b,
                                    idf[64:128, 64:128])
                nc.any.tensor_copy(rkrow[:], pb[0:1, 0:192])
                pbc = psx.tile([128, C], f32, tag="pbc")
                nc.tensor.matmul(pbc[:, :], ones1[:], rkrow[:], start=True, stop=True)
                nc.vector.tensor_tensor(L_a[:], L_a[:], pbc[:, :], Alu.mult)
                nc.vector.tensor_tensor(L_b[:], L_b[:], pbc[0:64, :], Alu.mult)
                filler2(L_a[:, 0:128])

                # full-row softmax; cross-head blocks masked to -1e30 -> exp 0.
                # a/b halves use independent scratch so their DVE/ACT ops
                # interleave instead of serializing.
                mx = small.tile([128, 1], f32, tag="mx")
                sm = small.tile([128, 1], f32, tag="sm")
                E = small.tile([128, C], f32, tag="E")
                mx2 = small.tile([64, 1], f32, tag="mx2")
                sm2 = small.tile([64, 1], f32, tag="sm2")
                E2 = small.tile([64, C], f32, tag="E2")
                nc.vector.tensor_tensor(L_a[:], L_a[:], mk_a[:], Alu.add)
                nc.vector.tensor_tensor(L_b[:], L_b[:], mk_b[:], Alu.add)
                nc.vector.tensor_reduce(mx[:], L_a[:], mybir.AxisListType.X, Alu.max)
                nc.vector.tensor_reduce(mx2[:], L_b[:], mybir.AxisListType.X, Alu.max)
                nc.vector.tensor_scalar_mul(mx[:], mx[:], -1.0)
                nc.vector.tensor_scalar_mul(mx2[:], mx2[:], -1.0)
                filler2(mx[:, :])
                nc.scalar.activation(E[:, :], L_a[:], Act.Exp,
                                     bias=mx[:], accum_out=sm[:])
                nc.scalar.activation(E2[:, :], L_b[:], Act.Exp,
                                     bias=mx2[:], accum_out=sm2[:])
                nc.vector.reciprocal(sm[:], sm[:])
                nc.vector.reciprocal(sm2[:], sm2[:])
                filler2(E[:, 0:128])
                nc.scalar.activation(attn_a[:, 0:C], E[:, :], Act.Copy,
                                     scale=sm[:])
                nc.scalar.activation(attn_b[0:64, 0:C], E2[:, :], Act.Copy,
                                     scale=sm2[:])
                filler2(E[:, 0:128])

                # fold projection into attention: FT[d, o] = sum_c attn[c,d] wp[c,o]
                pfa = psx.tile([128, C], f32, tag="pfa")
                pfb = psx.tile([128, C], f32, tag="pfb")
                nc.tensor.matmul(pfa[:, :], attn_a[:, 0:128], wp_a[:, :],
                                 start=True, stop=False)
                nc.tensor.matmul(pfa[:, :], attn_b[:, 0:128], wp_b[:, :],
                                 start=False, stop=True)
                nc.tensor.matmul(pfb[:, :], attn_a[:, 128:256], wp_a[:, :],
                                 start=True, stop=False)
                nc.tensor.matmul(pfb[:, :], attn_b[:, 128:256], wp_b[:, :],
                                 start=False, stop=True)
                nc.vector.tensor_copy(FT_a[:, 0:C], pfa[:, :])
                nc.scalar.copy(FT_b[0:64, 0:C], pfb[0:64, :])

            # ---- pass 2: out = FT.T @ v, streamed to HBM in bf16.
            #      Output staged in 8-row tiles so each DMA moves 256 KB. ----
            TB = 8
            with (
                tc.tile_pool(name="pout", bufs=4, space="PSUM") as pout,
                tc.tile_pool(name="o2", bufs=3) as opool,
            ):
                for blk in range(H // TB):
                    yb = blk * TB
                    ot_a = opool.tile([128, TB, W], bf16, tag="ota")
                    ot_b = opool.tile([64, TB, W], bf16, tag="otb")
                    for t in range(TB // 4):
                        r4 = yb + 4 * t
                        poa = pout.tile([128, 4, W], f32, tag="poa")
                        pob = pout.tile([128, 4, W], f32, tag="pob")
                        nc.tensor.matmul(poa[:, :, :], FT_a[:, 0:128],
                                         v_a[:, r4:r4 + 4, :], start=True, stop=False)
                        nc.tensor.matmul(poa[:, :, :], FT_b[:, 0:128],
                                         v_b[:, r4:r4 + 4, :], start=False, stop=True)
                        nc.tensor.matmul(pob[:, :, :], FT_a[:, 128:256],
                                         v_a[:, r4:r4 + 4, :], start=True, stop=False)
                        nc.tensor.matmul(pob[:, :, :], FT_b[:, 128:256],
                                         v_b[:, r4:r4 + 4, :], start=False, stop=True)
                        evac(ot_a[:, 4 * t:4 * t + 4, :], poa[:, :, :])
                        evac(ot_b[:, 4 * t:4 * t + 4, :], pob[0:64, :, :])
                    nc.sync.dma_start(out_d[0:128, yb:yb + TB, :], ot_a[:])
                    nc.sync.dma_start(out_d[128:192, yb:yb + TB, :], ot_b[:])

    nc.compile()
    return nc


# permuted channel order: tile0=q[0:128], tile1=k[0:128], tile2=v[0:128],
# tile3=q[128:192]+k[128:192], tile4=v[128:192]  (orig rows of w_qkv)
def _perm():
    return np.concatenate([
        np.arange(0, 128),          # q lo
        np.arange(192, 320),        # k lo
        np.arange(384, 512),        # v lo
        np.arange(128, 192),        # q hi
        np.arange(320, 384),        # k hi
        np.arange(512, 576),        # v hi
    ])


def _prep_weights(w_qkv, w_dw, w_project, temperature):
    perm = _perm()
    wqt = w_qkv[perm].T                                          # [192, 576]
    # pad to [128, 640]: wqa = input ch 0:128; wqb = input ch 128:192 + zeros
    wqa = np.zeros((128, C3P), np.float32)
    wqb = np.zeros((128, C3P), np.float32)
    for r in range(5):
        pn = PT5[r]
        wqa[:, 128 * r:128 * r + pn] = wqt[0:128, PO5[r]:PO5[r] + pn]
        wqb[0:64, 128 * r:128 * r + pn] = wqt[128:192, PO5[r]:PO5[r] + pn]
    # diagonal stationaries: block (r*9+k) = diag(w_dw[perm ch, k])
    wdg = np.zeros((128, 45, 128), np.float32)
    wd = w_dw.reshape(C3, 9)[perm]
    for r in range(5):
        po, pn = PO5[r], PT5[r]
        for k in range(9):
            blk = wdg[:, r * 9 + k, :]
            blk[np.arange(pn), np.arange(pn)] = wd[po:po + pn, k]
    wpt = w_project.T                                            # [c, o]
    wpa = wpt[0:128]
    wpb = np.zeros((128, C), np.float32)
    wpb[0:64] = wpt[128:192]
    tv = np.repeat(temperature.reshape(HEADS), DH).reshape(C, 1).astype(np.float32)
    mk = np.full((C, C), -1e30, np.float32)
    for h in range(HEADS):
        mk[h * DH:(h + 1) * DH, h * DH:(h + 1) * DH] = 0.0
    return {
        "wqa": wqa.astype(BF16),
        "wqb": wqb.astype(BF16),
        "wpa": np.ascontiguousarray(wpa).astype(BF16),
        "wpb": wpb.astype(BF16),
        "wdg": wdg.reshape(128, 45 * 128).astype(BF16),
        "tv": tv,
        "idf": np.eye(128, dtype=np.float32),
        "ones1": np.ones((1, 128), np.float32),
        "mka": mk[0:128],
        "mkb": mk[128:192],
    }


def kernel(x, w_qkv, w_dw, w_project, temperature, heads):
    from concourse import bass_utils

    x = np.asarray(x, np.float32)
    key = "nc"
    if key not in _CACHE:
        _CACHE[key] = _build()
    nc = _CACHE[key]

    shared = _prep_weights(np.asarray(w_qkv, np.float32),
                           np.asarray(w_dw, np.float32),
                           np.asarray(w_project, np.float32),
                           np.asarray(temperature, np.float32))
    in_maps = []
    for i in range(B):
        m = dict(shared)
        m["x0"] = x[i].reshape(C, H, W).astype(BF16)
        in_maps.append(m)

    res = bass_utils.run_bass_kernel_spmd(nc, in_maps, core_ids=list(range(B)))
    outs = [r["out0"].reshape(C, H, W) for r in res.results]
    return np.stack(outs, axis=0).astype(np.float32)


if __name__ == "__main__":
    rng = np.random.default_rng(0)
    x = rng.standard_normal((B, C, H, W)).astype(np.float32)
    w_qkv = (rng.standard_normal((C3, C)) / np.sqrt(C)).astype(np.float32)
    w_dw = (rng.standard_normal((C3, 1, 3, 3)) / 3.0).astype(np.float32)
    w_project = (rng.standard_normal((C, C)) / np.sqrt(C)).astype(np.float32)
    temperature = np.ones((HEADS, 1, 1), np.float32)
    y = kernel(x=x, w_qkv=w_qkv, w_dw=w_dw, w_project=w_project,
               temperature=temperature, heads=HEADS)
    print(y.shape, y.dtype)
